# revision 30
# baseline (speedup 1.0000x reference)
"""Depthwise 7x7 'same' conv (shared [K,K] kernel, scipy-style flipped) + mish,
on 8 Trainium2 NeuronCores, data-parallel over the batch axis.

Per core (2 batches x 64 ch = 128 planes of [256, 256]):
  - Conv decomposed per kernel-column v: y = sum_v T_v @ x_shift(v), where T_v
    is a [128,128] banded Toeplitz over H and the v-shift is a column offset
    into a W-padded slab in SBUF. fp32r tensor-engine mode (1 cyc/row, N=256).
  - Each plane is two 128-row blocks -> 14 accumulated fp32r matmuls into a
    per-plane 1-bank PSUM tile [128, 2, 256] (7 tiles in flight).
  - Rows 125..130 of each plane straddle the block boundary; a "seam" pass
    recomputes them exactly, batching 10 planes per matmul via a
    block-diagonal [120, 60] Toeplitz (runs early, fully overlapped).
  - mish(y) = y * (1 - 2/((E+1)^2 + 1)), E = exp(y):
      ACT: E = Exp(psum), F = Square(E + 1);  DVE: r = recip_approx_fast(d),
      out = psum * v'; d = F + 1 and v' = 1 - 2r rotate over Pool/DVE/ACT
      to balance engine load.
  - DMA: input slabs on the SP HWDGE queue, outputs on the ACT HWDGE queue,
    epilogue pipelined one supertile behind the matmuls.
"""
import sys

sys.path.insert(0, "/opt/trn_rl_repo")

import numpy as np

import concourse.bass as bass
import concourse.tile as tile
from concourse import bacc, mybir
from concourse.bass_utils import run_bass_kernel_spmd

AF = mybir.ActivationFunctionType
ALU = mybir.AluOpType

B, C, H, W = 16, 64, 256, 256
KK = 7
HALO = KK // 2            # 3
NCORES = 8
PLANES = (B // NCORES) * C  # 128 planes per core
WP = W + 2 * HALO           # 262 padded width
SEAM_G = 10                 # planes per seam matmul group
SEAM_ROWS = 2 * HALO * 2    # 12 input rows per plane (122..133)
SEAM_OUT = 2 * HALO         # 6 output rows per plane (125..130)
R0 = H // 2 - HALO          # 125: first seam output row
FP32R = True                # fast tensor-engine mode; flip to False for full fp32

_CACHE = {}


def _build_nc(reps=1):
    dt_mm = mybir.dt.float32r if FP32R else mybir.dt.float32
    nc = bacc.Bacc("TRN2", target_bir_lowering=False, debug=False)

    x = nc.dram_tensor("x", [PLANES, H, WP], dt_mm, kind="ExternalInput").ap()
    tmain = nc.dram_tensor("tmain", [128, KK, 128], dt_mm, kind="ExternalInput").ap()
    tseam = nc.dram_tensor(
        "tseam", [SEAM_G * SEAM_ROWS, KK, SEAM_G * SEAM_OUT], dt_mm, kind="ExternalInput"
    ).ap()
    y = nc.dram_tensor("y", [PLANES, H, W], mybir.dt.float32, kind="ExternalOutput").ap()

    n_groups = (PLANES + SEAM_G - 1) // SEAM_G  # 13 (12 full + one of 8)

    with tile.TileContext(nc) as tc:
        with (
            tc.tile_pool(name="consts", bufs=1) as consts,
            tc.tile_pool(name="xs", bufs=5) as xs,
            tc.tile_pool(name="ep", bufs=6) as ep,
            tc.tile_pool(name="res", bufs=5) as res,
            tc.tile_pool(name="ps", bufs=7, space="PSUM") as ps,
            tc.tile_pool(name="pseam", bufs=1, space="PSUM") as pseam,
        ):
            tm = consts.tile([128, KK, 128], dt_mm)
            nc.gpsimd.dma_start(out=tm, in_=tmain)
            tsm = consts.tile([SEAM_G * SEAM_ROWS, KK, SEAM_G * SEAM_OUT], dt_mm)
            nc.gpsimd.dma_start(out=tsm, in_=tseam)

            def rational_tail(E_flat, rows, tagsuf, shape_flat, d_eng="pool",
                              v_eng="pool"):
                """Given E = exp(y) (flat), return v' = 1 - 2/((E+1)^2+1)."""
                F = ep.tile(shape_flat, mybir.dt.float32, tag="F" + tagsuf)
                nc.scalar.activation(
                    out=F[:rows], in_=E_flat[:rows], func=AF.Square, bias=1.0
                )
                d_ = ep.tile(shape_flat, mybir.dt.float32, tag="d" + tagsuf)
                if d_eng == "act":
                    nc.scalar.activation(
                        out=d_[:rows], in_=F[:rows], func=AF.Identity, bias=1.0
                    )
                else:
                    nc.gpsimd.tensor_scalar_add(d_[:rows], F[:rows], 1.0)
                r_ = ep.tile(shape_flat, mybir.dt.float32, tag="r" + tagsuf)
                nc.vector.reciprocal_approx_fast(r_[:rows], d_[:rows])
                v_ = ep.tile(shape_flat, mybir.dt.float32, tag="v" + tagsuf)
                if v_eng == "act":
                    nc.scalar.activation(
                        out=v_[:rows], in_=r_[:rows], func=AF.Identity,
                        bias=1.0, scale=-2.0,
                    )
                else:
                    veng = nc.vector if v_eng == "dve" else nc.gpsimd
                    veng.tensor_scalar(
                        out=v_[:rows], in0=r_[:rows], scalar1=-2.0, scalar2=1.0,
                        op0=ALU.mult, op1=ALU.add,
                    )
                return v_

            def do_seam_group(g):
                p0 = g * SEAM_G
                gp = min(SEAM_G, PLANES - p0)
                kp = gp * SEAM_ROWS
                mp = gp * SEAM_OUT
                slab = xs.tile([SEAM_G * SEAM_ROWS, WP], dt_mm, tag="sseam")
                nc.sync.dma_start(
                    out=slab[:kp], in_=x[p0:p0 + gp, R0 - HALO:R0 - HALO + SEAM_ROWS, :]
                )
                psum = pseam.tile([SEAM_G * SEAM_OUT, W], mybir.dt.float32, tag="psseam")
                for v in range(KK):
                    nc.tensor.matmul(
                        psum[:mp], tsm[:kp, v, :mp], slab[:kp, v:v + W],
                        start=(v == 0), stop=(v == KK - 1),
                    )
                E = ep.tile([SEAM_G * SEAM_OUT, W], mybir.dt.float32, tag="Es")
                nc.scalar.activation(out=E[:mp], in_=psum[:mp], func=AF.Exp)
                v_ = rational_tail(E, mp, "s", [SEAM_G * SEAM_OUT, W])
                o_ = res.tile([SEAM_G * SEAM_OUT, W], mybir.dt.float32, tag="os")
                nc.vector.tensor_mul(o_[:mp], psum[:mp], v_[:mp])
                nc.scalar.dma_start(
                    out=y[p0:p0 + gp, R0:R0 + SEAM_OUT, :], in_=o_[:mp]
                )

            def do_epilogue(p, psums, sidx=0):
                o_ = res.tile([128, 2, 2, W], mybir.dt.float32, tag="om")
                for q in range(2):
                    k = 2 * sidx + q
                    pf = psums[q].rearrange("p b w -> p (b w)")
                    E = ep.tile([128, 2 * W], mybir.dt.float32, tag="Em")
                    nc.scalar.activation(out=E, in_=pf, func=AF.Exp)
                    v_ = rational_tail(
                        E, 128, "m", [128, 2 * W],
                        d_eng=("dve" if k % 4 == 3 else "pool"),
                        v_eng=("act" if k % 4 == 1 else "pool"),
                    )
                    nc.vector.tensor_mul(
                        o_[:, q].rearrange("p b w -> p (b w)"), pf, v_
                    )
                nc.scalar.dma_start(
                    out=y[p:p + 2, 0:R0, :].rearrange("q i w -> i q w"),
                    in_=o_[0:R0, :, 0, :],
                )
                nc.scalar.dma_start(
                    out=y[p:p + 2, R0 + SEAM_OUT:H, :].rearrange("q i w -> i q w"),
                    in_=o_[HALO:128, :, 1, :],
                )

            for _rep in range(reps):
                next_seam = 0
                pending = None
                for s in range(PLANES // 2):
                    p = 2 * s
                    slab = xs.tile([128, 2, 2, WP], dt_mm, tag="smain")
                    nc.sync.dma_start(
                        out=slab, in_=x[p:p + 2].rearrange("q (b r) w -> r q b w", b=2)
                    )
                    psums = []
                    for q in range(2):
                        psum = ps.tile([128, 2, W], mybir.dt.float32, tag="psmain")
                        for blk in range(2):
                            for v in range(KK):
                                nc.tensor.matmul(
                                    psum[:, blk, :], tm[:, v, :],
                                    slab[:, q, blk, v:v + W],
                                    start=(blk == 0 and v == 0),
                                    stop=(blk == 1 and v == KK - 1),
                                )
                        psums.append(psum)
                    if pending is not None:
                        do_epilogue(*pending)
                    pending = (p, psums, s)
                    while next_seam * SEAM_G <= p and next_seam < n_groups:
                        do_seam_group(next_seam)
                        next_seam += 1
                if pending is not None:
                    do_epilogue(*pending)
                while next_seam < n_groups:
                    do_seam_group(next_seam)
                    next_seam += 1

    nc.compile()
    return nc


def _toeplitz(kernel):
    kflip = np.asarray(kernel, np.float32)[::-1, ::-1].copy()
    k_i = np.arange(128)[:, None, None]
    v_i = np.arange(KK)[None, :, None]
    m_i = np.arange(128)[None, None, :]
    u = k_i - m_i + HALO
    tmain = np.where(
        (u >= 0) & (u < KK), kflip[np.clip(u, 0, KK - 1), v_i], np.float32(0)
    ).astype(np.float32)

    tseam = np.zeros((SEAM_G * SEAM_ROWS, KK, SEAM_G * SEAM_OUT), np.float32)
    blk = np.zeros((SEAM_ROWS, KK, SEAM_OUT), np.float32)
    for kk_ in range(SEAM_ROWS):
        for mm_ in range(SEAM_OUT):
            u2 = kk_ - mm_  # input row (122+kk) feeds output row (125+mm), tap u2
            if 0 <= u2 < KK:
                blk[kk_, :, mm_] = kflip[u2, :]
    for g in range(SEAM_G):
        tseam[g * SEAM_ROWS:(g + 1) * SEAM_ROWS, :, g * SEAM_OUT:(g + 1) * SEAM_OUT] = blk
    return tmain, tseam


def kernel(x, kernel):
    x = np.asarray(x, np.float32)
    ker = np.asarray(kernel, np.float32)
    assert x.shape == (B, C, H, W) and ker.shape == (KK, KK)

    if "nc" not in _CACHE:
        _CACHE["nc"] = _build_nc()
    nc = _CACHE["nc"]

    tmain, tseam = _toeplitz(ker)

    xp = np.zeros((B, C, H, WP), np.float32)
    xp[:, :, :, HALO:HALO + W] = x
    shards = xp.reshape(NCORES, PLANES, H, WP)

    in_maps = [
        {"x": shards[i], "tmain": tmain, "tseam": tseam} for i in range(NCORES)
    ]
    res = run_bass_kernel_spmd(nc, in_maps, list(range(NCORES)))
    out = np.concatenate(
        [res.results[i]["y"][None] for i in range(NCORES)], axis=0
    )
    return out.reshape(B, C, H, W)


# revision 38
# speedup vs baseline: 1.2564x; 1.2564x over previous
"""Depthwise 7x7 'same' conv (shared [K,K] kernel, scipy-style flipped) + mish,
on 8 Trainium2 NeuronCores, data-parallel over the batch axis.

Per core (2 batches x 64 ch = 128 planes of [256, 256]):
  - Conv decomposed per kernel-column v: y = sum_v T_v @ x_shift(v), where T_v
    is a [128,128] banded Toeplitz over H and the v-shift is a column offset
    into a W-padded slab in SBUF. fp32r tensor-engine mode (1 cyc/row, N=256).
  - Each plane is two 128-row blocks -> 14 accumulated fp32r matmuls into a
    per-plane 1-bank PSUM tile [128, 2, 256] (7 tiles in flight).
  - Rows 125..130 of each plane straddle the block boundary; a "seam" pass
    recomputes them exactly, batching 10 planes per matmul via a
    block-diagonal [120, 60] Toeplitz (runs early, fully overlapped).
  - mish(y) = y * (1 - 2/((E+1)^2 + 1)), E = exp(y):
      ACT: E = Exp(psum), F = Square(E + 1);  DVE: r = recip_approx_fast(d),
      out = psum * v'; d = F + 1 and v' = 1 - 2r rotate over Pool/DVE/ACT
      to balance engine load.
  - DMA: input slabs on the SP HWDGE queue, outputs on the ACT HWDGE queue,
    epilogue pipelined one supertile behind the matmuls.
"""
import sys

sys.path.insert(0, "/opt/trn_rl_repo")

import numpy as np

import concourse.bass as bass
import concourse.tile as tile
from concourse import bacc, mybir
from concourse.bass_utils import run_bass_kernel_spmd

AF = mybir.ActivationFunctionType
ALU = mybir.AluOpType

B, C, H, W = 16, 64, 256, 256
KK = 7
HALO = KK // 2            # 3
NCORES = 8
PLANES = (B // NCORES) * C  # 128 planes per core
WP = W + 2 * HALO           # 262 padded width
SEAM_G = 10                 # planes per seam matmul group
SEAM_ROWS = 2 * HALO * 2    # 12 input rows per plane (122..133)
SEAM_OUT = 2 * HALO         # 6 output rows per plane (125..130)
R0 = H // 2 - HALO          # 125: first seam output row
FP32R = True                # fast tensor-engine mode; flip to False for full fp32

_CACHE = {}


def _build_nc(reps=1):
    dt_mm = mybir.dt.float32r if FP32R else mybir.dt.float32
    nc = bacc.Bacc("TRN2", target_bir_lowering=False, debug=False)

    x = nc.dram_tensor("x", [PLANES, H, WP], dt_mm, kind="ExternalInput").ap()
    tmain = nc.dram_tensor("tmain", [128, KK, 128], dt_mm, kind="ExternalInput").ap()
    tseam = nc.dram_tensor(
        "tseam", [SEAM_G * SEAM_ROWS, KK, SEAM_G * SEAM_OUT], dt_mm, kind="ExternalInput"
    ).ap()
    y = nc.dram_tensor("y", [PLANES, H, W], mybir.dt.float32, kind="ExternalOutput").ap()

    n_groups = (PLANES + SEAM_G - 1) // SEAM_G  # 13 (12 full + one of 8)

    with tile.TileContext(nc) as tc:
        with (
            tc.tile_pool(name="consts", bufs=1) as consts,
            tc.tile_pool(name="xs", bufs=5) as xs,
            tc.tile_pool(name="ep", bufs=6) as ep,
            tc.tile_pool(name="res", bufs=5) as res,
            tc.tile_pool(name="ps", bufs=7, space="PSUM") as ps,
            tc.tile_pool(name="pseam", bufs=1, space="PSUM") as pseam,
        ):
            tm = consts.tile([128, KK, 128], dt_mm)
            nc.gpsimd.dma_start(out=tm, in_=tmain)
            tsm = consts.tile([SEAM_G * SEAM_ROWS, KK, SEAM_G * SEAM_OUT], dt_mm)
            nc.gpsimd.dma_start(out=tsm, in_=tseam)

            def rational_tail(E_flat, rows, tagsuf, shape_flat, d_eng="pool",
                              v_eng="pool"):
                """Given E = exp(y) (flat), return v' = 1 - 2/((E+1)^2+1)."""
                F = ep.tile(shape_flat, mybir.dt.float32, tag="F" + tagsuf)
                nc.scalar.activation(
                    out=F[:rows], in_=E_flat[:rows], func=AF.Square, bias=1.0
                )
                d_ = ep.tile(shape_flat, mybir.dt.float32, tag="d" + tagsuf)
                if d_eng == "act":
                    nc.scalar.activation(
                        out=d_[:rows], in_=F[:rows], func=AF.Identity, bias=1.0
                    )
                else:
                    nc.gpsimd.tensor_scalar_add(d_[:rows], F[:rows], 1.0)
                r_ = ep.tile(shape_flat, mybir.dt.float32, tag="r" + tagsuf)
                nc.vector.reciprocal_approx_fast(r_[:rows], d_[:rows])
                v_ = ep.tile(shape_flat, mybir.dt.float32, tag="v" + tagsuf)
                if v_eng == "act":
                    nc.scalar.activation(
                        out=v_[:rows], in_=r_[:rows], func=AF.Identity,
                        bias=1.0, scale=-2.0,
                    )
                else:
                    veng = nc.vector if v_eng == "dve" else nc.gpsimd
                    veng.tensor_scalar(
                        out=v_[:rows], in0=r_[:rows], scalar1=-2.0, scalar2=1.0,
                        op0=ALU.mult, op1=ALU.add,
                    )
                return v_

            def do_seam_group(g):
                p0 = g * SEAM_G
                gp = min(SEAM_G, PLANES - p0)
                kp = gp * SEAM_ROWS
                mp = gp * SEAM_OUT
                slab = xs.tile([SEAM_G * SEAM_ROWS, WP], dt_mm, tag="sseam")
                nc.sync.dma_start(
                    out=slab[:kp], in_=x[p0:p0 + gp, R0 - HALO:R0 - HALO + SEAM_ROWS, :]
                )
                psum = pseam.tile([SEAM_G * SEAM_OUT, W], mybir.dt.float32, tag="psseam")
                for v in range(KK):
                    nc.tensor.matmul(
                        psum[:mp], tsm[:kp, v, :mp], slab[:kp, v:v + W],
                        start=(v == 0), stop=(v == KK - 1),
                    )
                E = ep.tile([SEAM_G * SEAM_OUT, W], mybir.dt.float32, tag="Es")
                nc.scalar.activation(out=E[:mp], in_=psum[:mp], func=AF.Exp)
                v_ = rational_tail(E, mp, "s", [SEAM_G * SEAM_OUT, W])
                o_ = res.tile([SEAM_G * SEAM_OUT, W], mybir.dt.float32, tag="os")
                nc.vector.tensor_mul(o_[:mp], psum[:mp], v_[:mp])
                nc.scalar.dma_start(
                    out=y[p0:p0 + gp, R0:R0 + SEAM_OUT, :], in_=o_[:mp]
                )

            def do_epilogue(p, psums, sidx=0):
                o_ = res.tile([128, 2, 2, W], mybir.dt.float32, tag="om")
                for q in range(2):
                    k = 2 * sidx + q
                    pf = psums[q].rearrange("p b w -> p (b w)")
                    E = ep.tile([128, 2 * W], mybir.dt.float32, tag="Em")
                    nc.scalar.activation(out=E, in_=pf, func=AF.Exp)
                    v_ = rational_tail(
                        E, 128, "m", [128, 2 * W],
                        d_eng=("dve" if k % 4 == 3 else "pool"),
                        v_eng=("act" if k % 4 == 1 else "pool"),
                    )
                    nc.vector.tensor_mul(
                        o_[:, q].rearrange("p b w -> p (b w)"), pf, v_
                    )
                nc.scalar.dma_start(
                    out=y[p:p + 2, 0:R0, :].rearrange("q i w -> i q w"),
                    in_=o_[0:R0, :, 0, :],
                )
                nc.scalar.dma_start(
                    out=y[p:p + 2, R0 + SEAM_OUT:H, :].rearrange("q i w -> i q w"),
                    in_=o_[HALO:128, :, 1, :],
                )

            for _rep in range(reps):
                next_seam = 0
                pending = []
                for s in range(PLANES // 2):
                    p = 2 * s
                    slab = xs.tile([128, 2, 2, WP], dt_mm, tag="smain")
                    nc.sync.dma_start(
                        out=slab, in_=x[p:p + 2].rearrange("q (b r) w -> r q b w", b=2)
                    )
                    psums = []
                    for q in range(2):
                        psum = ps.tile([128, 2, W], mybir.dt.float32, tag="psmain")
                        for blk in range(2):
                            for v in range(KK):
                                nc.tensor.matmul(
                                    psum[:, blk, :], tm[:, v, :],
                                    slab[:, q, blk, v:v + W],
                                    start=(blk == 0 and v == 0),
                                    stop=(blk == 1 and v == KK - 1),
                                )
                        psums.append(psum)
                    pending.append((p, psums, s))
                    if len(pending) > 2:
                        do_epilogue(*pending.pop(0))
                    while next_seam * SEAM_G <= p and next_seam < n_groups:
                        do_seam_group(next_seam)
                        next_seam += 1
                while pending:
                    do_epilogue(*pending.pop(0))
                while next_seam < n_groups:
                    do_seam_group(next_seam)
                    next_seam += 1

    nc.compile()
    return nc


def _toeplitz(kernel):
    kflip = np.asarray(kernel, np.float32)[::-1, ::-1].copy()
    k_i = np.arange(128)[:, None, None]
    v_i = np.arange(KK)[None, :, None]
    m_i = np.arange(128)[None, None, :]
    u = k_i - m_i + HALO
    tmain = np.where(
        (u >= 0) & (u < KK), kflip[np.clip(u, 0, KK - 1), v_i], np.float32(0)
    ).astype(np.float32)

    tseam = np.zeros((SEAM_G * SEAM_ROWS, KK, SEAM_G * SEAM_OUT), np.float32)
    blk = np.zeros((SEAM_ROWS, KK, SEAM_OUT), np.float32)
    for kk_ in range(SEAM_ROWS):
        for mm_ in range(SEAM_OUT):
            u2 = kk_ - mm_  # input row (122+kk) feeds output row (125+mm), tap u2
            if 0 <= u2 < KK:
                blk[kk_, :, mm_] = kflip[u2, :]
    for g in range(SEAM_G):
        tseam[g * SEAM_ROWS:(g + 1) * SEAM_ROWS, :, g * SEAM_OUT:(g + 1) * SEAM_OUT] = blk
    return tmain, tseam


def kernel(x, kernel):
    x = np.asarray(x, np.float32)
    ker = np.asarray(kernel, np.float32)
    assert x.shape == (B, C, H, W) and ker.shape == (KK, KK)

    if "nc" not in _CACHE:
        _CACHE["nc"] = _build_nc()
    nc = _CACHE["nc"]

    tmain, tseam = _toeplitz(ker)

    xp = np.zeros((B, C, H, WP), np.float32)
    xp[:, :, :, HALO:HALO + W] = x
    shards = xp.reshape(NCORES, PLANES, H, WP)

    in_maps = [
        {"x": shards[i], "tmain": tmain, "tseam": tseam} for i in range(NCORES)
    ]
    res = run_bass_kernel_spmd(nc, in_maps, list(range(NCORES)))
    out = np.concatenate(
        [res.results[i]["y"][None] for i in range(NCORES)], axis=0
    )
    return out.reshape(B, C, H, W)
